# revision 54
# baseline (speedup 1.0000x reference)
"""Trainium2 Bass kernel for nn_Matcher (retrieval_knn attention).

Math (per object o, with S=1 batch):
  logits[b,n] = (keys[o,:,b] . q_in[:,n]) / sqrt(Dk)
  p           = softmax_b(logits)
  mem[v,n]    = sum_b values[o,v,b] p[b,n]
  maskmem[n]  = sum_b masks[o,b] p[b,n]
  out[o]      = concat([mem, q_out * maskmem], axis=0)   # [1024, n]

Sharding: 8 cores = 4 objects x 2 query halves (n in [0,1800) / [1800,3600)).
No cross-core communication.

Per-core kernel (v11, 113.0us TimelineSim vs the 137.9us v4 baseline; the
container's fake-NRT "hardware" time IS the TimelineSim cost model, so the
layout is tuned against instruction_cost_v2.rs):

  Phase A (4 x 450-query strips): mm1 = keys^T q_in in BF16 (halves DMA
      traffic vs fp32r at the same 1 cyc/col PE rate; +~0.05% rel err) into
      four single-bank PSUM slots; exp per 450-chunk split DVE (Schraudolph
      fp8 bit-trick via tensor_scalar, 594ns) : ACT (true exp, 555ns) =
      109:119 by deficit round-robin -- phase A is exp-engine-bound, so the
      PE is topped up with inline DoubleRow streams: strip 0 runs the
      mask/ones sweep (tiny weights -- no vals-DMA dependency), strips 1-3
      run their m0 value streams, and strip 0's m0 trickles through strips
      1-3 (m00_fill) once its acc bank frees.  Strip 0's normalization
      chain (md2 copy -> bc broadcasts -> 1/denom) also runs here.
  PE p-state warmup: 5 dummy matmuls burn the initial DMA wait so the
      cost model's 0.65/1.2/2.4GHz ramp (100ns/3us continuous-busy
      thresholds) is done before the real stream starts.
  DMA schedule: every dma_start pays a serial 625ns HWDGE hold plus a
      single shared ~340GB/s DMA_ENGINES device, so transfers are few,
      large, and ordered by first use: keys c0-1, qin strip 0, keys groups
      [6,12,16,21] (mo after group 0; qin rest + e8-pad zeros + sel2 after
      group 2), then vals (first read in strip 1), qout last.
  Phase B: mo sweeps for strips 1-3 (pure DR back-to-back, md2 copies
      emitted at each stop), then m1[0]/m2[0] bridge the normalization
      chains (their acc0 frees first), norm chains + m0 norms, the qout
      gating path on the otherwise-idle Pool engine (SBUF-only operands,
      batched 2-in/4-out DMAs), then the remaining value passes ordered so
      every pass's accumulator bank is freed by a norm-mul that already
      happened: (1,1),(3,0),(1,2),(1,3),(2,1),(2,2),(2,3),(3,1),(3,2),(3,3).
  Normalization in bf16 throughout (outputs are bf16 anyway).

Cost-model facts this layout exploits (instruction_cost_v2.rs):
  - matmul: out_free_size x pe_cycle x {bf16/fp32r: 1.0, fp8e4+DoubleRow:
    0.5} cycles; Ldweights is FREE (0ns engine time, so no ldweights
    dedup/regroup post-passes are needed); matmul PSUM outputs only at
    base partitions 0/32/64 (walrus ISA check; 96 and odd offsets reject).
  - DVE/ACT: (max access_cycles + free_size) * cycle_t with access PSUM/
    SBUF = 120/58 (DVE, 0.96GHz), 172/222 (ACT, 1.2GHz); DVE 2x/4x modes
    need all-SBUF (and 2-byte for 4x) operands -- impossible for exp
    (PSUM input), used implicitly by the Pool qout muls.
  - Engine SEQs are in-order with one wait slot per instruction (extra
    deps emit blocking EventSemaphores): emission order is preserved, so
    never emit a PE instruction that waits on a slow cross-engine chain
    ahead of ready PE work (head-of-line blocking).
  - PSUM is the scarce resource: 8 banks = 4 s-slots + 4 accumulators;
    deeper s (wider exp instructions amortize the access overhead) was
    always beaten by losing inline-DR accumulator banks.
"""

import sys

sys.path.insert(0, "/opt/trn_rl_repo")

import numpy as np
import ml_dtypes

OBJ_N, D_KEY, D_VAL, BANK_N, N_Q = 4, 128, 512, 7200, 3600
N_CORES = 8
N_HALF = N_Q // 2            # 1800 queries per core
P = 128
NB = (BANK_N + P - 1) // P   # 57 bank chunks (56 x 128 + 1 x 32)
NB2 = NB + 1                 # 58: padded to even for DoubleRow pairs
NPAIR = NB2 // 2             # 29
B_PAD = NB2 * P              # 7424
LAST_BW = BANK_N - (NB - 1) * P  # 32
SCALE = 1.0 / float(np.sqrt(D_KEY))
SHIFT = 2.5                  # exp(z-SHIFT): keeps e in fp8_e4m3 range
NW = 450                     # strip width (4 x 450 = 1800)
NWP = 1824                   # e8 row pitch (full 1800 padded to %16==0)
DVE_EXP = True               # DVE slots: Schraudolph fp8-exp
A_EXP = float(SCALE * 8.0 / np.log(2.0))
B_EXP = float(56.0 - 8.0 * SHIFT / np.log(2.0) + 0.35)
N_CHUNKS = [(i * NW, NW) for i in range(4)]
DMA_GROUP = 8                # bank chunks per bulk DMA
VAL_GROUP = 8                # value pairs per bulk DMA
# exp engine split (cost model: DVE 450-col = (120+450)*1.0417 = 594ns,
# ACT = (216+450)*0.8333 = 555ns): DVE takes 110 of the 228 slots.
N_SLOTS = 4 * NB
N_DVE = 110
DVE_SLOTS = frozenset(
    i for i in range(N_SLOTS)
    if (i * N_DVE) // N_SLOTS != ((i + 1) * N_DVE) // N_SLOTS)

_CACHE = {}


def _build(reps=1, bench=False, reload_in_rep=True):
    import concourse.bacc as bacc
    import concourse.mybir as mybir
    import concourse.tile as tile

    f32 = mybir.dt.float32
    bf16 = mybir.dt.bfloat16
    f8 = mybir.dt.float8e4
    u8 = mybir.dt.uint8
    Exp = mybir.ActivationFunctionType.Exp
    DR = mybir.MatmulPerfMode.DoubleRow

    nc = bacc.Bacc("TRN2", target_bir_lowering=False, debug=False)

    ikind = {} if bench else {"kind": "ExternalInput"}
    okind = {} if bench else {"kind": "ExternalOutput"}
    consts_d = nc.dram_tensor("consts", [2, P], bf16, kind="ExternalInput")
    keys_d = nc.dram_tensor("keys", [D_KEY, NB * P], bf16, **ikind)
    vals_d = nc.dram_tensor("vals", [P, NPAIR * 4 * 2 * P], u8, **ikind)
    mo_d = nc.dram_tensor("mo", [P, NPAIR * 2 * 16], u8, **ikind)
    zeros_d = nc.dram_tensor("zeros", [P, NWP], u8, kind="ExternalInput")
    qin_d = nc.dram_tensor("qin", [D_KEY, N_HALF], bf16, **ikind)
    qout_d = nc.dram_tensor("qout", [D_VAL, N_HALF], bf16, **ikind)
    out_d = nc.dram_tensor("out", [2 * D_VAL, N_HALF], bf16, **okind)
    if bench:
        dout_d = nc.dram_tensor("dout", [1, P], bf16, kind="ExternalOutput")

    keys_ap = keys_d.ap().rearrange("d (c q) -> d c q", q=P)        # [128, 57, 128]
    vals_ap = vals_d.ap().rearrange("p (c m q) -> p c m q",
                                    c=NPAIR, m=4)        # [128, 29, 4, 256]
    mo_ap = mo_d.ap().rearrange("p (c j) -> p c j", c=NPAIR)   # [128, 29, 32]
    qout_ap = qout_d.ap().rearrange("(r p) n -> p r n", p=P)        # [128, 4, 1800]
    out_ap = out_d.ap().rearrange("(r p) n -> p r n", p=P)          # [128, 8, 1800]

    with tile.TileContext(nc) as tc:
        with (
            tc.tile_pool(name="persist", bufs=1) as persist,
            tc.tile_pool(name="qin_p", bufs=2) as qin_p,
            tc.tile_pool(name="qout_p", bufs=1) as qout_p,
            tc.tile_pool(name="row_p", bufs=2) as row_p,
            tc.tile_pool(name="bcsb_p", bufs=2) as bcsb_p,
            tc.tile_pool(name="out_p", bufs=6) as out_p,
            tc.tile_pool(name="s_ps", bufs=2, space="PSUM") as s_ps,
            tc.tile_pool(name="acc_ps", bufs=1, space="PSUM") as acc_ps,
        ):
            # Persistent operands
            keys_sb = persist.tile([P, NB, P], bf16)
            vals_sb = persist.tile([P, NPAIR, 4, 2 * P], u8)
            vals_f8 = vals_sb.bitcast(f8)
            mo_sb = persist.tile([P, NPAIR, 32], u8)
            mo_f8 = mo_sb.bitcast(f8)
            e8_sb = persist.tile([P, NB2, NWP], f8)
            e8_u8 = e8_sb.bitcast(u8)
            ones_col = persist.tile([1, P], bf16)
            nc.vector.memset(ones_col[:], 1.0)
            shift_sb = persist.tile([P, 1], f32)
            nc.vector.memset(shift_sb[:], -SHIFT)
            # Warm the ACT exp table so the first real exp doesn't pay the
            # ACT_TABLE_LOAD on the critical path.
            warm = persist.tile([1, 1], f32)
            nc.vector.memset(warm[:], 0.0)
            nc.scalar.activation(warm[:], warm[:], Exp, scale=1.0)
            sel2 = persist.tile([2, P], bf16)  # row0=0, row1=1 (selects denom)
            # PE p-state warmup: the cost model runs matmuls at 0.65/1.2GHz
            # until 100ns/3us of continuous PE busy.  Burn the initial DMA
            # wait (~3.2us) on dependency-free dummy matmuls so the real
            # stream starts at 2.4GHz.
            wrow = persist.tile([1, 512], bf16)
            nc.gpsimd.memset(wrow[:], 1.0)
            for w in range(5):
                wt = s_ps.tile([P, 512], f32, tag="s", bufs=4,
                               padded_shape=[P, 512], name=f"warmpe_{w}")
                nc.tensor.matmul(wt[:, :512], ones_col[:1, :], wrow[:, :512],
                                 start=True, stop=True)

            def bulk_load(first_rep):
                """DMA schedule: every dma_start pays a serial 625ns HWDGE
                hold, so order by first-use and batch aggressively.
                qin strip 0 + first keys land ~2.4us in; zeros/sel2 slot
                into the gap before the back keys groups; vals (only read
                from strip 1 on) go last."""
                qin_t0 = qin_p.tile([P, N_HALF], bf16, tag="qin", name="qin_t0")
                if first_rep:
                    nc.sync.dma_start(keys_sb[:, 0:2, :], keys_ap[:, 0:2, :])
                nc.sync.dma_start(qin_t0[:, 0:NW], qin_d.ap()[:, 0:NW])
                g0 = 2
                for gi, gsz in enumerate([6, 12, 16, 21]):
                    g1 = min(g0 + gsz, NB)
                    if first_rep:
                        nc.sync.dma_start(keys_sb[:, g0:g1, :], keys_ap[:, g0:g1, :])
                    g0 = g1
                    if gi == 0 and first_rep:
                        nc.sync.dma_start(mo_sb[:], mo_ap[:, :, :])
                    if gi == 2:
                        nc.sync.dma_start(qin_t0[:, NW:], qin_d.ap()[:, NW:])
                    if gi == 3:
                        if first_rep:
                            # e8 pad zeroing (read by every strip's pair-28
                            # DR; engines can't address partition offsets)
                            nc.sync.dma_start(
                                e8_sb[:, NB2 - 1:NB2, :].bitcast(u8),
                                zeros_d.ap()[:, 0:NWP])
                            nc.sync.dma_start(
                                e8_sb[LAST_BW:, NB - 1:NB, :].bitcast(u8),
                                zeros_d.ap()[LAST_BW:, 0:NWP])
                            nc.sync.dma_start(sel2[:], consts_d.ap()[:, :])
                if first_rep:
                    p0 = 0
                    while p0 < NPAIR:
                        p1 = min(p0 + VAL_GROUP, NPAIR)
                        nc.sync.dma_start(vals_sb[:, p0:p1, :, :],
                                          vals_ap[:, p0:p1, :, :])
                        p0 = p1
                return qin_t0

            for _rep in range(reps):
                qin_t = bulk_load(reload_in_rep or _rep == 0)

                def valw(pq, m):
                    return vals_f8[:, pq, m, :].rearrange("k (i q) -> k i q", i=2)

                def mow(pq):
                    return mo_f8[:, pq, :].rearrange(
                        "k (i j) -> k i j", i=2)[:, :, 0:2]

                # Phase A: mm1 + exp per 450-chunk + inline DR stream per
                # strip (strip 0: mask/ones; strips 1-3: m0).
                acc_in = [acc_ps.tile([P, NW], f32, tag=f"acc{j}",
                                      name=f"accA_{j}", padded_shape=[P, 512])
                          for j in range(4)]
                slot_i = [0]

                def exp_chunk(c, j, n0, nw, bw):
                    s_t = s_ps.tile([P, NW], f32, tag="s", bufs=4,
                                    padded_shape=[P, 512], name=f"s_{c}_{j}")
                    nc.tensor.matmul(
                        s_t[:bw, :nw], keys_sb[:, c, :bw],
                        qin_t[:, n0:n0 + nw], start=True, stop=True,
                    )
                    if DVE_EXP and slot_i[0] in DVE_SLOTS:
                        nc.vector.tensor_scalar(
                            e8_u8[:bw, c:c + 1, n0:n0 + nw],
                            s_t[:bw, :nw], A_EXP, B_EXP,
                            op0=mybir.AluOpType.mult,
                            op1=mybir.AluOpType.add)
                    else:
                        nc.scalar.activation(
                            e8_sb[:bw, c:c + 1, n0:n0 + nw],
                            s_t[:bw, :nw],
                            Exp, scale=SCALE, bias=shift_sb[:bw, :])
                    slot_i[0] += 1

                rbs = [None] * 4
                mns = [None] * 4

                def norm_md2(j, md2_src):
                    """PSUM mask/denom rows -> bf16 SBUF (ACT copy). Emit as
                    early as possible; the ACT queue drains it behind exps."""
                    md2 = row_p.tile([2, NW], bf16, tag="md2", bufs=4,
                                     name=f"md2_{j}")
                    nc.scalar.copy(md2[:], md2_src)
                    return md2

                def norm_rest(j, n0, nw, md2):
                    """bc1/bc2 broadcasts (PE) -> rb, mn (DVE, bf16). Emit
                    well after norm_md2 so PE never head-of-line blocks on
                    the ACT copy."""
                    bc1 = s_ps.tile([P, NW], f32, tag="s", bufs=4,
                                    name=f"bc1_{j}", padded_shape=[P, 512])
                    nc.tensor.matmul(bc1[:, :nw], sel2[:], md2[:],
                                     start=True, stop=True)
                    rb_sb = bcsb_p.tile([P, nw], bf16, tag=f"rb{j}",
                                        name=f"rb_{j}", bufs=1)
                    with nc.allow_low_precision(
                            reason="outputs are bf16 anyway; 1/denom in bf16 "
                                   "adds <0.4% on top of bf16 out rounding"):
                        nc.vector.reciprocal(rb_sb[:], bc1[:, :nw])
                    bc2 = s_ps.tile([P, NW], f32, tag="s", bufs=4,
                                    name=f"bc2_{j}", padded_shape=[P, 512])
                    nc.tensor.matmul(bc2[:, :nw], ones_col[:], md2[0:1, :],
                                     start=True, stop=True)
                    mn_sb = bcsb_p.tile([P, nw], bf16, tag=f"mn{j}",
                                        name=f"mn_{j}", bufs=1)
                    nc.vector.tensor_mul(mn_sb[:], bc2[:, :nw], rb_sb[:])
                    rbs[j] = rb_sb
                    mns[j] = mn_sb

                md2s = [None] * 4
                m0_acc0 = [None]
                m00_pq = [0]

                m1_acc0 = [None]
                m10_pq = [0]

                def mpass0_fill(acc, mq, counter, frac):
                    """Spread strip 0's m-pass DRs through later strips as
                    PE filler (m0 over strips 1-2, m1 over strip 3 once m0's
                    norm-mul freed the bank mid-phase-A)."""
                    target = min(NPAIR, int(frac * NPAIR) + 1)
                    n0, nw = N_CHUNKS[0]
                    while counter[0] < target:
                        pq = counter[0]
                        nc.tensor.matmul(
                            acc[0][:, :nw], valw(pq, mq),
                            e8_sb[:, 2 * pq:2 * pq + 2, n0:n0 + nw],
                            start=(pq == 0), stop=(pq == NPAIR - 1),
                            perf_mode=DR,
                        )
                        counter[0] += 1

                for j, (n0, nw) in enumerate(N_CHUNKS):
                    for pc in range(NPAIR + 1):
                        if pc < NPAIR:
                            c0 = 2 * pc
                            exp_chunk(c0, j, n0, nw, P)
                            if c0 + 1 < NB:
                                bw1 = P if c0 + 1 < NB - 1 else LAST_BW
                                exp_chunk(c0 + 1, j, n0, nw, bw1)
                        if pc > 0:
                            pq = pc - 1
                            if j == 0:
                                nc.tensor.matmul(
                                    acc_in[0][0:2, :nw], mow(pq),
                                    e8_sb[:, 2 * pq:2 * pq + 2, n0:n0 + nw],
                                    start=(pc == 1), stop=(pc == NPAIR),
                                    perf_mode=DR,
                                )
                            else:
                                nc.tensor.matmul(
                                    acc_in[j][:, :nw], valw(pq, 0),
                                    e8_sb[:, 2 * pq:2 * pq + 2, n0:n0 + nw],
                                    start=(pc == 1), stop=(pc == NPAIR),
                                    perf_mode=DR,
                                )
                        if j in (1, 2):
                            mpass0_fill(m0_acc0, 0, m00_pq,
                                        ((j - 1) * (NPAIR + 1) + pc) /
                                        (2.0 * (NPAIR + 1) - 2))
                        elif j == 3:
                            mpass0_fill(m1_acc0, 1, m10_pq,
                                        pc / float(NPAIR))
                        if j == 1 and pc == 3:
                            # strip 0's normalization broadcasts, deferred a
                            # few steps so PE never waits on the md2 copy
                            norm_rest(0, *N_CHUNKS[0], md2s[0])
                    if j == 0:
                        md2s[0] = norm_md2(0, acc_in[0][0:2, :NW])
                        m0_acc0[0] = acc_ps.tile([P, NW], f32, tag="acc0",
                                                 name="accm0_0",
                                                 padded_shape=[P, 512])
                    elif j == 2:
                        # m0[0] complete: normalize now so its bank frees
                        # for the inline m1[0] stream during strip 3
                        n00, nw0 = N_CHUNKS[0]
                        o_t = out_p.tile([P, nw0], bf16, tag="out")
                        nc.vector.tensor_mul(o_t[:], m0_acc0[0][:, :nw0],
                                             rbs[0])
                        nc.sync.dma_start(out_ap[:, 0, n00:n00 + nw0], o_t[:])
                        m1_acc0[0] = acc_ps.tile([P, NW], f32, tag="acc0",
                                                 name="accm1_0",
                                                 padded_shape=[P, 512])

                # Phase B: all mo sweeps first (pure DR back-to-back on PE,
                # with each strip's md2 copy emitted at its sweep's stop so
                # the ACT copies drain under the following sweeps), then the
                # norm chains, m0 norm muls, qout path, m1..m3.
                (n0, nw) = N_CHUNKS[0]
                o_t = out_p.tile([P, nw], bf16, tag="out")
                nc.vector.tensor_mul(o_t[:], m1_acc0[0][:, :nw], rbs[0])
                nc.sync.dma_start(out_ap[:, 1, n0:n0 + nw], o_t[:])

                for j in (1, 2, 3):
                    n0, nw = N_CHUNKS[j]
                    acc5_t = s_ps.tile([P, NW], f32, tag="s", bufs=4,
                                       name=f"acc5_{j}", padded_shape=[P, 512])
                    for pq in range(NPAIR):
                        nc.tensor.matmul(
                            acc5_t[0:2, :nw], mow(pq),
                            e8_sb[:, 2 * pq:2 * pq + 2, n0:n0 + nw],
                            start=(pq == 0), stop=(pq == NPAIR - 1),
                            perf_mode=DR,
                        )
                    md2s[j] = norm_md2(j, acc5_t[0:2, :nw])

                # m1/m2 strip 0 bridge the norm-chain region (acc0 freed
                # by m0[0]'s norm mul at phase-B start, then by m1[0]'s)
                def val_pass(m, j, bridge_tag=None):
                    n0, nw = N_CHUNKS[j]
                    acc_t = acc_ps.tile([P, NW], f32,
                                        tag=bridge_tag or f"acc{j}",
                                        name=f"accm{m}_{j}",
                                        padded_shape=[P, 512])
                    for pq in range(NPAIR):
                        nc.tensor.matmul(
                            acc_t[:, :nw], valw(pq, m),
                            e8_sb[:, 2 * pq:2 * pq + 2, n0:n0 + nw],
                            start=(pq == 0), stop=(pq == NPAIR - 1),
                            perf_mode=DR,
                        )
                    o_t = out_p.tile([P, nw], bf16, tag="out")
                    nc.vector.tensor_mul(o_t[:], acc_t[:, :nw], rbs[j])
                    nc.sync.dma_start(out_ap[:, m, n0:n0 + nw], o_t[:])

                val_pass(2, 0)

                for j in (1, 2, 3):
                    n0, nw = N_CHUNKS[j]
                    norm_rest(j, n0, nw, md2s[j])
                    # m0[j] accumulated inline in phase A: normalize now.
                    o_t = out_p.tile([P, nw], bf16, tag="out")
                    nc.vector.tensor_mul(o_t[:], acc_in[j][:, :nw], rbs[j])
                    nc.sync.dma_start(out_ap[:, 0, n0:n0 + nw], o_t[:])

                # qout gating path on the Pool engine (SBUF-only operands),
                # batched IO: 2 input DMAs, one output DMA per strip.
                for h in (0, 1):
                    qout_t = qout_p.tile([P, D_VAL // P, 2 * NW], bf16,
                                         tag="qout", bufs=2)
                    nc.sync.dma_start(
                        qout_t[:], qout_ap[:, :, 2 * h * NW:2 * (h + 1) * NW])
                    for jj in (0, 1):
                        j = 2 * h + jj
                        n0, nw = N_CHUNKS[j]
                        o_t = out_p.tile([P, 4, NW], bf16, tag="outq", bufs=2)
                        for m in range(4):
                            nc.gpsimd.tensor_mul(
                                o_t[:, m, :],
                                qout_t[:, m, jj * NW:jj * NW + nw], mns[j])
                        nc.sync.dma_start(out_ap[:, 4:8, n0:n0 + nw], o_t[:])

                for m, j in [(1, 1), (3, 0), (1, 2), (1, 3),
                             (2, 1), (2, 2), (2, 3),
                             (3, 1), (3, 2), (3, 3)]:
                    val_pass(m, j)

            if bench:
                dsb = persist.tile([1, P], bf16)
                nc.vector.tensor_copy(dsb[:], ones_col[:])
                nc.sync.dma_start(dout_d.ap()[:, :], dsb[:])

    nc.compile()
    return nc


def _get_nc():
    if "nc" not in _CACHE:
        _CACHE["nc"] = _build()
    return _CACHE["nc"]


def _get_runner():
    """Build the multi-core PJRT runner once (mirrors bass2jax.run_bass_via_pjrt)."""
    if "runner" in _CACHE:
        return _CACHE["runner"]
    import jax
    from jax.sharding import Mesh, PartitionSpec
    from jax.experimental.shard_map import shard_map
    import concourse.mybir as mybir
    from concourse import bass2jax
    from concourse.bass2jax import _bass_exec_p, install_neuronx_cc_hook

    nc = _get_nc()
    install_neuronx_cc_hook()
    partition_name = nc.partition_id_tensor.name if nc.partition_id_tensor else None
    in_names, out_names, out_avals = [], [], []
    for alloc in nc.m.functions[0].allocations:
        if not isinstance(alloc, mybir.MemoryLocationSet):
            continue
        name = alloc.memorylocations[0].name
        if alloc.kind == "ExternalInput":
            if name != partition_name:
                in_names.append(name)
        elif alloc.kind == "ExternalOutput":
            out_names.append(name)
            out_avals.append(jax.core.ShapedArray(
                tuple(alloc.tensor_shape), mybir.dt.np(alloc.dtype)))
    n_params = len(in_names)
    zero_outs = [np.zeros(a.shape, a.dtype) for a in out_avals]
    all_in_names = list(in_names) + list(out_names)
    if partition_name is not None:
        all_in_names.append(partition_name)

    def _body(*args):
        operands = list(args)
        if partition_name is not None:
            operands.append(bass2jax.partition_id_tensor())
        outs = _bass_exec_p.bind(
            *operands,
            out_avals=tuple(out_avals),
            in_names=tuple(all_in_names),
            out_names=tuple(out_names),
            lowering_input_output_aliases=(),
            sim_require_finite=True,
            sim_require_nnan=True,
            nc=nc,
        )
        return tuple(outs)

    try:
        devices = jax.devices("axon")
    except Exception:
        devices = [d for d in jax.devices() if d.platform != "cpu"] or jax.devices()
    devices = devices[:N_CORES]
    assert len(devices) >= N_CORES, f"need {N_CORES} cores, got {len(devices)}"
    mesh = Mesh(np.asarray(devices), ("core",))
    n_io = n_params + len(out_names)
    fn = jax.jit(
        shard_map(_body, mesh=mesh,
                  in_specs=(PartitionSpec("core"),) * n_io,
                  out_specs=(PartitionSpec("core"),) * len(out_names),
                  check_rep=False),
        keep_unused=True)

    def run(in_maps):
        concat_in = [
            np.concatenate([np.asarray(m[name]) for m in in_maps], axis=0)
            for name in in_names
        ]
        concat_zero = [
            np.zeros((N_CORES * z.shape[0], *z.shape[1:]), z.dtype)
            for z in zero_outs
        ]
        out_arrs = fn(*concat_in, *concat_zero)
        return [
            {name: np.asarray(out_arrs[i]).reshape(N_CORES, *out_avals[i].shape)[c]
             for i, name in enumerate(out_names)}
            for c in range(N_CORES)
        ]

    _CACHE["runner"] = run
    return run


def kernel(keys, values, masks, q_in, q_out):

    keys = np.ascontiguousarray(np.asarray(keys, dtype=np.float32))
    values = np.asarray(values, dtype=np.float32)
    masks = np.asarray(masks, dtype=np.float32)
    q_in = np.ascontiguousarray(np.asarray(q_in, dtype=np.float32))
    q_out = np.asarray(q_out, dtype=np.float32)

    f8 = ml_dtypes.float8_e4m3
    bf = ml_dtypes.bfloat16

    # Host-side layout prep (per object, shared by 2 cores)
    keys_pad = np.zeros((OBJ_N, D_KEY, NB * P), dtype=bf)
    keys_pad[:, :, :BANK_N] = keys.astype(bf)
    vpad = np.zeros((OBJ_N, D_VAL, B_PAD), dtype=f8)
    vpad[:, :, :BANK_N] = values.astype(f8)
    a = vpad.reshape(OBJ_N, 4, P, NPAIR, 2, P)   # [o, m, q, pc, i, p]
    # vals8[o, p, pc, m, i, q] = values[o, m*128+q, (2*pc+i)*128 + p]
    vals8 = a.transpose(0, 5, 3, 1, 4, 2)
    vals8 = np.ascontiguousarray(vals8).reshape(OBJ_N, P, NPAIR * 4 * 2 * P)
    vals8 = vals8.view(np.uint8)
    mpad = np.zeros((OBJ_N, 2, B_PAD), dtype=f8)
    mpad[:, 0, :BANK_N] = masks[:, 0].astype(f8)
    mpad[:, 1, :BANK_N] = 1.0
    mr = mpad.reshape(OBJ_N, 2, NPAIR, 2, P)     # [o, row(0=mask,1=ones), pc, i, p]
    mo8 = np.zeros((OBJ_N, P, NPAIR, 32), dtype=f8)
    # [pc, i, j] blocks at stride 16: j=0 mask, j=1 ones
    mo8.reshape(OBJ_N, P, NPAIR, 2, 16)[:, :, :, :, 0:2] = (
        mr.transpose(0, 4, 2, 3, 1))
    mo8 = np.ascontiguousarray(mo8).reshape(OBJ_N, P, NPAIR * 32)
    mo8 = mo8.view(np.uint8)
    zeros8 = np.zeros((P, NWP), dtype=np.uint8)

    consts = np.zeros((2, P), dtype=bf)
    consts[1, :] = 1.0
    q_in_bf = q_in.astype(bf)
    q_out_bf = q_out.astype(bf)

    in_maps = []
    for core in range(N_CORES):
        o, half = divmod(core, 2)
        nsl = slice(half * N_HALF, (half + 1) * N_HALF)
        in_maps.append({
            "consts": consts,
            "keys": keys_pad[o],
            "vals": vals8[o],
            "mo": mo8[o],
            "zeros": zeros8,
            "qin": np.ascontiguousarray(q_in_bf[0, :, nsl]),
            "qout": np.ascontiguousarray(q_out_bf[0, :, nsl]),
        })

    run = _get_runner()
    results = run(in_maps)

    out = np.empty((1, OBJ_N, 2 * D_VAL, N_Q), dtype=np.float32)
    for core in range(N_CORES):
        o, half = divmod(core, 2)
        nsl = slice(half * N_HALF, (half + 1) * N_HALF)
        out[0, o, :, nsl] = results[core]["out"].astype(np.float32)
    return out


# revision 55
# speedup vs baseline: 1.0071x; 1.0071x over previous
"""Trainium2 Bass kernel for nn_Matcher (retrieval_knn attention).

Math (per object o, with S=1 batch):
  logits[b,n] = (keys[o,:,b] . q_in[:,n]) / sqrt(Dk)
  p           = softmax_b(logits)
  mem[v,n]    = sum_b values[o,v,b] p[b,n]
  maskmem[n]  = sum_b masks[o,b] p[b,n]
  out[o]      = concat([mem, q_out * maskmem], axis=0)   # [1024, n]

Sharding: 8 cores = 4 objects x 2 query halves (n in [0,1800) / [1800,3600)).
No cross-core communication.

Per-core kernel (v11, 113.0us TimelineSim vs the 137.9us v4 baseline; the
container's fake-NRT "hardware" time IS the TimelineSim cost model, so the
layout is tuned against instruction_cost_v2.rs):

  Phase A (4 x 450-query strips): mm1 = keys^T q_in in BF16 (halves DMA
      traffic vs fp32r at the same 1 cyc/col PE rate; +~0.05% rel err) into
      four single-bank PSUM slots; exp per 450-chunk split DVE (Schraudolph
      fp8 bit-trick via tensor_scalar, 594ns) : ACT (true exp, 555ns) =
      109:119 by deficit round-robin -- phase A is exp-engine-bound, so the
      PE is topped up with inline DoubleRow streams: strip 0 runs the
      mask/ones sweep (tiny weights -- no vals-DMA dependency), strips 1-3
      run their m0 value streams, and strip 0's m0 trickles through strips
      1-3 (m00_fill) once its acc bank frees.  Strip 0's normalization
      chain (md2 copy -> bc broadcasts -> 1/denom) also runs here.
  PE p-state warmup: 5 dummy matmuls burn the initial DMA wait so the
      cost model's 0.65/1.2/2.4GHz ramp (100ns/3us continuous-busy
      thresholds) is done before the real stream starts.
  DMA schedule: every dma_start pays a serial 625ns HWDGE hold plus a
      single shared ~340GB/s DMA_ENGINES device, so transfers are few,
      large, and ordered by first use: keys c0-1, qin strip 0, keys groups
      [6,12,16,21] (mo after group 0; qin rest + e8-pad zeros + sel2 after
      group 2), then vals (first read in strip 1), qout last.
  Phase B: mo sweeps for strips 1-3 (pure DR back-to-back, md2 copies
      emitted at each stop), then m1[0]/m2[0] bridge the normalization
      chains (their acc0 frees first), norm chains + m0 norms, the qout
      gating path on the otherwise-idle Pool engine (SBUF-only operands,
      batched 2-in/4-out DMAs), then the remaining value passes ordered so
      every pass's accumulator bank is freed by a norm-mul that already
      happened: (1,1),(3,0),(1,2),(1,3),(2,1),(2,2),(2,3),(3,1),(3,2),(3,3).
  Normalization in bf16 throughout (outputs are bf16 anyway).

Cost-model facts this layout exploits (instruction_cost_v2.rs):
  - matmul: out_free_size x pe_cycle x {bf16/fp32r: 1.0, fp8e4+DoubleRow:
    0.5} cycles; Ldweights is FREE (0ns engine time, so no ldweights
    dedup/regroup post-passes are needed); matmul PSUM outputs only at
    base partitions 0/32/64 (walrus ISA check; 96 and odd offsets reject).
  - DVE/ACT: (max access_cycles + free_size) * cycle_t with access PSUM/
    SBUF = 120/58 (DVE, 0.96GHz), 172/222 (ACT, 1.2GHz); DVE 2x/4x modes
    need all-SBUF (and 2-byte for 4x) operands -- impossible for exp
    (PSUM input), used implicitly by the Pool qout muls.
  - Engine SEQs are in-order with one wait slot per instruction (extra
    deps emit blocking EventSemaphores): emission order is preserved, so
    never emit a PE instruction that waits on a slow cross-engine chain
    ahead of ready PE work (head-of-line blocking).
  - PSUM is the scarce resource: 8 banks = 4 s-slots + 4 accumulators;
    deeper s (wider exp instructions amortize the access overhead) was
    always beaten by losing inline-DR accumulator banks.
"""

import sys

sys.path.insert(0, "/opt/trn_rl_repo")

import numpy as np
import ml_dtypes

OBJ_N, D_KEY, D_VAL, BANK_N, N_Q = 4, 128, 512, 7200, 3600
N_CORES = 8
N_HALF = N_Q // 2            # 1800 queries per core
P = 128
NB = (BANK_N + P - 1) // P   # 57 bank chunks (56 x 128 + 1 x 32)
NB2 = NB + 1                 # 58: padded to even for DoubleRow pairs
NPAIR = NB2 // 2             # 29
B_PAD = NB2 * P              # 7424
LAST_BW = BANK_N - (NB - 1) * P  # 32
SCALE = 1.0 / float(np.sqrt(D_KEY))
SHIFT = 2.5                  # exp(z-SHIFT): keeps e in fp8_e4m3 range
NW = 450                     # strip width (4 x 450 = 1800)
NWP = 1824                   # e8 row pitch (full 1800 padded to %16==0)
DVE_EXP = True               # DVE slots: Schraudolph fp8-exp
A_EXP = float(SCALE * 8.0 / np.log(2.0))
B_EXP = float(56.0 - 8.0 * SHIFT / np.log(2.0) + 0.35)
N_CHUNKS = [(i * NW, NW) for i in range(4)]
DMA_GROUP = 8                # bank chunks per bulk DMA
VAL_GROUP = 8                # value pairs per bulk DMA
# exp engine split (cost model: DVE 450-col = (120+450)*1.0417 = 594ns,
# ACT = (216+450)*0.8333 = 555ns): DVE takes 110 of the 228 slots.
N_SLOTS = 4 * NB
N_DVE = 109
DVE_SLOTS = frozenset(
    i for i in range(N_SLOTS)
    if (i * N_DVE) // N_SLOTS != ((i + 1) * N_DVE) // N_SLOTS)

_CACHE = {}


def _build(reps=1, bench=False, reload_in_rep=True):
    import concourse.bacc as bacc
    import concourse.mybir as mybir
    import concourse.tile as tile

    f32 = mybir.dt.float32
    bf16 = mybir.dt.bfloat16
    f8 = mybir.dt.float8e4
    u8 = mybir.dt.uint8
    Exp = mybir.ActivationFunctionType.Exp
    DR = mybir.MatmulPerfMode.DoubleRow

    nc = bacc.Bacc("TRN2", target_bir_lowering=False, debug=False)

    ikind = {} if bench else {"kind": "ExternalInput"}
    okind = {} if bench else {"kind": "ExternalOutput"}
    consts_d = nc.dram_tensor("consts", [2, P], bf16, kind="ExternalInput")
    keys_d = nc.dram_tensor("keys", [D_KEY, NB * P], bf16, **ikind)
    vals_d = nc.dram_tensor("vals", [P, NPAIR * 4 * 2 * P], u8, **ikind)
    mo_d = nc.dram_tensor("mo", [P, NPAIR * 2 * 16], u8, **ikind)
    zeros_d = nc.dram_tensor("zeros", [P, NWP], u8, kind="ExternalInput")
    qin_d = nc.dram_tensor("qin", [D_KEY, N_HALF], bf16, **ikind)
    qout_d = nc.dram_tensor("qout", [D_VAL, N_HALF], bf16, **ikind)
    out_d = nc.dram_tensor("out", [2 * D_VAL, N_HALF], bf16, **okind)
    if bench:
        dout_d = nc.dram_tensor("dout", [1, P], bf16, kind="ExternalOutput")

    keys_ap = keys_d.ap().rearrange("d (c q) -> d c q", q=P)        # [128, 57, 128]
    vals_ap = vals_d.ap().rearrange("p (c m q) -> p c m q",
                                    c=NPAIR, m=4)        # [128, 29, 4, 256]
    mo_ap = mo_d.ap().rearrange("p (c j) -> p c j", c=NPAIR)   # [128, 29, 32]
    qout_ap = qout_d.ap().rearrange("(r p) n -> p r n", p=P)        # [128, 4, 1800]
    out_ap = out_d.ap().rearrange("(r p) n -> p r n", p=P)          # [128, 8, 1800]

    with tile.TileContext(nc) as tc:
        with (
            tc.tile_pool(name="persist", bufs=1) as persist,
            tc.tile_pool(name="qin_p", bufs=2) as qin_p,
            tc.tile_pool(name="qout_p", bufs=1) as qout_p,
            tc.tile_pool(name="row_p", bufs=2) as row_p,
            tc.tile_pool(name="bcsb_p", bufs=2) as bcsb_p,
            tc.tile_pool(name="out_p", bufs=6) as out_p,
            tc.tile_pool(name="s_ps", bufs=2, space="PSUM") as s_ps,
            tc.tile_pool(name="acc_ps", bufs=1, space="PSUM") as acc_ps,
        ):
            # Persistent operands
            keys_sb = persist.tile([P, NB, P], bf16)
            vals_sb = persist.tile([P, NPAIR, 4, 2 * P], u8)
            vals_f8 = vals_sb.bitcast(f8)
            mo_sb = persist.tile([P, NPAIR, 32], u8)
            mo_f8 = mo_sb.bitcast(f8)
            e8_sb = persist.tile([P, NB2, NWP], f8)
            e8_u8 = e8_sb.bitcast(u8)
            ones_col = persist.tile([1, P], bf16)
            nc.vector.memset(ones_col[:], 1.0)
            shift_sb = persist.tile([P, 1], f32)
            nc.vector.memset(shift_sb[:], -SHIFT)
            # Warm the ACT exp table so the first real exp doesn't pay the
            # ACT_TABLE_LOAD on the critical path.
            warm = persist.tile([1, 1], f32)
            nc.vector.memset(warm[:], 0.0)
            nc.scalar.activation(warm[:], warm[:], Exp, scale=1.0)
            sel2 = persist.tile([2, P], bf16)  # row0=0, row1=1 (selects denom)
            # PE p-state warmup: the cost model runs matmuls at 0.65/1.2GHz
            # until 100ns/3us of continuous PE busy.  Burn the initial DMA
            # wait (~3.2us) on dependency-free dummy matmuls so the real
            # stream starts at 2.4GHz.
            wrow = persist.tile([1, 512], bf16)
            nc.gpsimd.memset(wrow[:], 1.0)
            for w in range(5):
                wt = s_ps.tile([P, 512], f32, tag="s", bufs=4,
                               padded_shape=[P, 512], name=f"warmpe_{w}")
                nc.tensor.matmul(wt[:, :512], ones_col[:1, :], wrow[:, :512],
                                 start=True, stop=True)

            def bulk_load(first_rep):
                """DMA schedule: every dma_start pays a serial 625ns HWDGE
                hold, so order by first-use and batch aggressively.
                qin strip 0 + first keys land ~2.4us in; zeros/sel2 slot
                into the gap before the back keys groups; vals (only read
                from strip 1 on) go last."""
                qin_t0 = qin_p.tile([P, N_HALF], bf16, tag="qin", name="qin_t0")
                if first_rep:
                    nc.sync.dma_start(keys_sb[:, 0:2, :], keys_ap[:, 0:2, :])
                nc.sync.dma_start(qin_t0[:, 0:NW], qin_d.ap()[:, 0:NW])
                g0 = 2
                for gi, gsz in enumerate([6, 12, 16, 21]):
                    g1 = min(g0 + gsz, NB)
                    if first_rep:
                        nc.sync.dma_start(keys_sb[:, g0:g1, :], keys_ap[:, g0:g1, :])
                    g0 = g1
                    if gi == 0 and first_rep:
                        nc.sync.dma_start(mo_sb[:], mo_ap[:, :, :])
                    if gi == 2:
                        nc.sync.dma_start(qin_t0[:, NW:], qin_d.ap()[:, NW:])
                    if gi == 3:
                        if first_rep:
                            # e8 pad zeroing (read by every strip's pair-28
                            # DR; engines can't address partition offsets)
                            nc.sync.dma_start(
                                e8_sb[:, NB2 - 1:NB2, :].bitcast(u8),
                                zeros_d.ap()[:, 0:NWP])
                            nc.sync.dma_start(
                                e8_sb[LAST_BW:, NB - 1:NB, :].bitcast(u8),
                                zeros_d.ap()[LAST_BW:, 0:NWP])
                            nc.sync.dma_start(sel2[:], consts_d.ap()[:, :])
                if first_rep:
                    p0 = 0
                    while p0 < NPAIR:
                        p1 = min(p0 + VAL_GROUP, NPAIR)
                        nc.sync.dma_start(vals_sb[:, p0:p1, :, :],
                                          vals_ap[:, p0:p1, :, :])
                        p0 = p1
                return qin_t0

            for _rep in range(reps):
                qin_t = bulk_load(reload_in_rep or _rep == 0)

                def valw(pq, m):
                    return vals_f8[:, pq, m, :].rearrange("k (i q) -> k i q", i=2)

                def mow(pq):
                    return mo_f8[:, pq, :].rearrange(
                        "k (i j) -> k i j", i=2)[:, :, 0:2]

                # Phase A: mm1 + exp per 450-chunk + inline DR stream per
                # strip (strip 0: mask/ones; strips 1-3: m0).
                acc_in = [acc_ps.tile([P, NW], f32, tag=f"acc{j}",
                                      name=f"accA_{j}", padded_shape=[P, 512])
                          for j in range(4)]
                slot_i = [0]

                def exp_chunk(c, j, n0, nw, bw):
                    s_t = s_ps.tile([P, NW], f32, tag="s", bufs=4,
                                    padded_shape=[P, 512], name=f"s_{c}_{j}")
                    nc.tensor.matmul(
                        s_t[:bw, :nw], keys_sb[:, c, :bw],
                        qin_t[:, n0:n0 + nw], start=True, stop=True,
                    )
                    if DVE_EXP and slot_i[0] in DVE_SLOTS:
                        nc.vector.tensor_scalar(
                            e8_u8[:bw, c:c + 1, n0:n0 + nw],
                            s_t[:bw, :nw], A_EXP, B_EXP,
                            op0=mybir.AluOpType.mult,
                            op1=mybir.AluOpType.add)
                    else:
                        nc.scalar.activation(
                            e8_sb[:bw, c:c + 1, n0:n0 + nw],
                            s_t[:bw, :nw],
                            Exp, scale=SCALE, bias=shift_sb[:bw, :])
                    slot_i[0] += 1

                rbs = [None] * 4
                mns = [None] * 4

                def norm_md2(j, md2_src):
                    """PSUM mask/denom rows -> bf16 SBUF (ACT copy). Emit as
                    early as possible; the ACT queue drains it behind exps."""
                    md2 = row_p.tile([2, NW], bf16, tag="md2", bufs=4,
                                     name=f"md2_{j}")
                    nc.scalar.copy(md2[:], md2_src)
                    return md2

                def norm_rest(j, n0, nw, md2):
                    """bc1/bc2 broadcasts (PE) -> rb, mn (DVE, bf16). Emit
                    well after norm_md2 so PE never head-of-line blocks on
                    the ACT copy."""
                    bc1 = s_ps.tile([P, NW], f32, tag="s", bufs=4,
                                    name=f"bc1_{j}", padded_shape=[P, 512])
                    nc.tensor.matmul(bc1[:, :nw], sel2[:], md2[:],
                                     start=True, stop=True)
                    rb_sb = bcsb_p.tile([P, nw], bf16, tag=f"rb{j}",
                                        name=f"rb_{j}", bufs=1)
                    with nc.allow_low_precision(
                            reason="outputs are bf16 anyway; 1/denom in bf16 "
                                   "adds <0.4% on top of bf16 out rounding"):
                        nc.vector.reciprocal(rb_sb[:], bc1[:, :nw])
                    bc2 = s_ps.tile([P, NW], f32, tag="s", bufs=4,
                                    name=f"bc2_{j}", padded_shape=[P, 512])
                    nc.tensor.matmul(bc2[:, :nw], ones_col[:], md2[0:1, :],
                                     start=True, stop=True)
                    mn_sb = bcsb_p.tile([P, nw], bf16, tag=f"mn{j}",
                                        name=f"mn_{j}", bufs=1)
                    nc.vector.tensor_mul(mn_sb[:], bc2[:, :nw], rb_sb[:])
                    rbs[j] = rb_sb
                    mns[j] = mn_sb

                md2s = [None] * 4
                m0_acc0 = [None]
                m00_pq = [0]

                m1_acc0 = [None]
                m10_pq = [0]

                def mpass0_fill(acc, mq, counter, frac):
                    """Spread strip 0's m-pass DRs through later strips as
                    PE filler (m0 over strips 1-2, m1 over strip 3 once m0's
                    norm-mul freed the bank mid-phase-A)."""
                    target = min(NPAIR, int(frac * NPAIR) + 1)
                    n0, nw = N_CHUNKS[0]
                    while counter[0] < target:
                        pq = counter[0]
                        nc.tensor.matmul(
                            acc[0][:, :nw], valw(pq, mq),
                            e8_sb[:, 2 * pq:2 * pq + 2, n0:n0 + nw],
                            start=(pq == 0), stop=(pq == NPAIR - 1),
                            perf_mode=DR,
                        )
                        counter[0] += 1

                for j, (n0, nw) in enumerate(N_CHUNKS):
                    for pc in range(NPAIR + 1):
                        if pc < NPAIR:
                            c0 = 2 * pc
                            exp_chunk(c0, j, n0, nw, P)
                            if c0 + 1 < NB:
                                bw1 = P if c0 + 1 < NB - 1 else LAST_BW
                                exp_chunk(c0 + 1, j, n0, nw, bw1)
                        if pc > 0:
                            pq = pc - 1
                            if j == 0:
                                nc.tensor.matmul(
                                    acc_in[0][0:2, :nw], mow(pq),
                                    e8_sb[:, 2 * pq:2 * pq + 2, n0:n0 + nw],
                                    start=(pc == 1), stop=(pc == NPAIR),
                                    perf_mode=DR,
                                )
                            else:
                                nc.tensor.matmul(
                                    acc_in[j][:, :nw], valw(pq, 0),
                                    e8_sb[:, 2 * pq:2 * pq + 2, n0:n0 + nw],
                                    start=(pc == 1), stop=(pc == NPAIR),
                                    perf_mode=DR,
                                )
                        if j in (1, 2):
                            mpass0_fill(m0_acc0, 0, m00_pq,
                                        ((j - 1) * (NPAIR + 1) + pc) /
                                        (2.0 * (NPAIR + 1) - 2))
                        elif j == 3:
                            mpass0_fill(m1_acc0, 1, m10_pq,
                                        pc / float(NPAIR))
                        if j == 1 and pc == 3:
                            # strip 0's normalization broadcasts, deferred a
                            # few steps so PE never waits on the md2 copy
                            norm_rest(0, *N_CHUNKS[0], md2s[0])
                    if j == 0:
                        md2s[0] = norm_md2(0, acc_in[0][0:2, :NW])
                        m0_acc0[0] = acc_ps.tile([P, NW], f32, tag="acc0",
                                                 name="accm0_0",
                                                 padded_shape=[P, 512])
                    elif j == 2:
                        # m0[0] complete: normalize now so its bank frees
                        # for the inline m1[0] stream during strip 3
                        n00, nw0 = N_CHUNKS[0]
                        o_t = out_p.tile([P, nw0], bf16, tag="out")
                        nc.vector.tensor_mul(o_t[:], m0_acc0[0][:, :nw0],
                                             rbs[0])
                        nc.sync.dma_start(out_ap[:, 0, n00:n00 + nw0], o_t[:])
                        m1_acc0[0] = acc_ps.tile([P, NW], f32, tag="acc0",
                                                 name="accm1_0",
                                                 padded_shape=[P, 512])

                # Phase B: all mo sweeps first (pure DR back-to-back on PE,
                # with each strip's md2 copy emitted at its sweep's stop so
                # the ACT copies drain under the following sweeps), then the
                # norm chains, m0 norm muls, qout path, m1..m3.
                (n0, nw) = N_CHUNKS[0]
                o_t = out_p.tile([P, nw], bf16, tag="out")
                nc.vector.tensor_mul(o_t[:], m1_acc0[0][:, :nw], rbs[0])
                nc.sync.dma_start(out_ap[:, 1, n0:n0 + nw], o_t[:])

                for j in (1, 2, 3):
                    n0, nw = N_CHUNKS[j]
                    acc5_t = s_ps.tile([P, NW], f32, tag="s", bufs=4,
                                       name=f"acc5_{j}", padded_shape=[P, 512])
                    for pq in range(NPAIR):
                        nc.tensor.matmul(
                            acc5_t[0:2, :nw], mow(pq),
                            e8_sb[:, 2 * pq:2 * pq + 2, n0:n0 + nw],
                            start=(pq == 0), stop=(pq == NPAIR - 1),
                            perf_mode=DR,
                        )
                    md2s[j] = norm_md2(j, acc5_t[0:2, :nw])

                # m1/m2 strip 0 bridge the norm-chain region (acc0 freed
                # by m0[0]'s norm mul at phase-B start, then by m1[0]'s)
                def val_pass(m, j, bridge_tag=None):
                    n0, nw = N_CHUNKS[j]
                    acc_t = acc_ps.tile([P, NW], f32,
                                        tag=bridge_tag or f"acc{j}",
                                        name=f"accm{m}_{j}",
                                        padded_shape=[P, 512])
                    for pq in range(NPAIR):
                        nc.tensor.matmul(
                            acc_t[:, :nw], valw(pq, m),
                            e8_sb[:, 2 * pq:2 * pq + 2, n0:n0 + nw],
                            start=(pq == 0), stop=(pq == NPAIR - 1),
                            perf_mode=DR,
                        )
                    o_t = out_p.tile([P, nw], bf16, tag="out")
                    nc.vector.tensor_mul(o_t[:], acc_t[:, :nw], rbs[j])
                    nc.sync.dma_start(out_ap[:, m, n0:n0 + nw], o_t[:])

                val_pass(2, 0)

                for j in (1, 2, 3):
                    n0, nw = N_CHUNKS[j]
                    norm_rest(j, n0, nw, md2s[j])
                    # m0[j] accumulated inline in phase A: normalize now.
                    o_t = out_p.tile([P, nw], bf16, tag="out")
                    nc.vector.tensor_mul(o_t[:], acc_in[j][:, :nw], rbs[j])
                    nc.sync.dma_start(out_ap[:, 0, n0:n0 + nw], o_t[:])

                # qout gating path on the Pool engine (SBUF-only operands),
                # batched IO: 2 input DMAs, one output DMA per strip.
                for h in (0, 1):
                    qout_t = qout_p.tile([P, D_VAL // P, 2 * NW], bf16,
                                         tag="qout", bufs=2)
                    nc.sync.dma_start(
                        qout_t[:], qout_ap[:, :, 2 * h * NW:2 * (h + 1) * NW])
                    for jj in (0, 1):
                        j = 2 * h + jj
                        n0, nw = N_CHUNKS[j]
                        o_t = out_p.tile([P, 4, NW], bf16, tag="outq", bufs=2)
                        for m in range(4):
                            nc.gpsimd.tensor_mul(
                                o_t[:, m, :],
                                qout_t[:, m, jj * NW:jj * NW + nw], mns[j])
                        nc.sync.dma_start(out_ap[:, 4:8, n0:n0 + nw], o_t[:])

                for m, j in [(1, 1), (3, 0), (1, 2), (1, 3),
                             (2, 1), (2, 2), (2, 3),
                             (3, 1), (3, 2), (3, 3)]:
                    val_pass(m, j)

            if bench:
                dsb = persist.tile([1, P], bf16)
                nc.vector.tensor_copy(dsb[:], ones_col[:])
                nc.sync.dma_start(dout_d.ap()[:, :], dsb[:])

    nc.compile()
    return nc


def _get_nc():
    if "nc" not in _CACHE:
        _CACHE["nc"] = _build()
    return _CACHE["nc"]


def _get_runner():
    """Build the multi-core PJRT runner once (mirrors bass2jax.run_bass_via_pjrt)."""
    if "runner" in _CACHE:
        return _CACHE["runner"]
    import jax
    from jax.sharding import Mesh, PartitionSpec
    from jax.experimental.shard_map import shard_map
    import concourse.mybir as mybir
    from concourse import bass2jax
    from concourse.bass2jax import _bass_exec_p, install_neuronx_cc_hook

    nc = _get_nc()
    install_neuronx_cc_hook()
    partition_name = nc.partition_id_tensor.name if nc.partition_id_tensor else None
    in_names, out_names, out_avals = [], [], []
    for alloc in nc.m.functions[0].allocations:
        if not isinstance(alloc, mybir.MemoryLocationSet):
            continue
        name = alloc.memorylocations[0].name
        if alloc.kind == "ExternalInput":
            if name != partition_name:
                in_names.append(name)
        elif alloc.kind == "ExternalOutput":
            out_names.append(name)
            out_avals.append(jax.core.ShapedArray(
                tuple(alloc.tensor_shape), mybir.dt.np(alloc.dtype)))
    n_params = len(in_names)
    zero_outs = [np.zeros(a.shape, a.dtype) for a in out_avals]
    all_in_names = list(in_names) + list(out_names)
    if partition_name is not None:
        all_in_names.append(partition_name)

    def _body(*args):
        operands = list(args)
        if partition_name is not None:
            operands.append(bass2jax.partition_id_tensor())
        outs = _bass_exec_p.bind(
            *operands,
            out_avals=tuple(out_avals),
            in_names=tuple(all_in_names),
            out_names=tuple(out_names),
            lowering_input_output_aliases=(),
            sim_require_finite=True,
            sim_require_nnan=True,
            nc=nc,
        )
        return tuple(outs)

    try:
        devices = jax.devices("axon")
    except Exception:
        devices = [d for d in jax.devices() if d.platform != "cpu"] or jax.devices()
    devices = devices[:N_CORES]
    assert len(devices) >= N_CORES, f"need {N_CORES} cores, got {len(devices)}"
    mesh = Mesh(np.asarray(devices), ("core",))
    n_io = n_params + len(out_names)
    fn = jax.jit(
        shard_map(_body, mesh=mesh,
                  in_specs=(PartitionSpec("core"),) * n_io,
                  out_specs=(PartitionSpec("core"),) * len(out_names),
                  check_rep=False),
        keep_unused=True)

    def run(in_maps):
        concat_in = [
            np.concatenate([np.asarray(m[name]) for m in in_maps], axis=0)
            for name in in_names
        ]
        concat_zero = [
            np.zeros((N_CORES * z.shape[0], *z.shape[1:]), z.dtype)
            for z in zero_outs
        ]
        out_arrs = fn(*concat_in, *concat_zero)
        return [
            {name: np.asarray(out_arrs[i]).reshape(N_CORES, *out_avals[i].shape)[c]
             for i, name in enumerate(out_names)}
            for c in range(N_CORES)
        ]

    _CACHE["runner"] = run
    return run


def kernel(keys, values, masks, q_in, q_out):

    keys = np.ascontiguousarray(np.asarray(keys, dtype=np.float32))
    values = np.asarray(values, dtype=np.float32)
    masks = np.asarray(masks, dtype=np.float32)
    q_in = np.ascontiguousarray(np.asarray(q_in, dtype=np.float32))
    q_out = np.asarray(q_out, dtype=np.float32)

    f8 = ml_dtypes.float8_e4m3
    bf = ml_dtypes.bfloat16

    # Host-side layout prep (per object, shared by 2 cores)
    keys_pad = np.zeros((OBJ_N, D_KEY, NB * P), dtype=bf)
    keys_pad[:, :, :BANK_N] = keys.astype(bf)
    vpad = np.zeros((OBJ_N, D_VAL, B_PAD), dtype=f8)
    vpad[:, :, :BANK_N] = values.astype(f8)
    a = vpad.reshape(OBJ_N, 4, P, NPAIR, 2, P)   # [o, m, q, pc, i, p]
    # vals8[o, p, pc, m, i, q] = values[o, m*128+q, (2*pc+i)*128 + p]
    vals8 = a.transpose(0, 5, 3, 1, 4, 2)
    vals8 = np.ascontiguousarray(vals8).reshape(OBJ_N, P, NPAIR * 4 * 2 * P)
    vals8 = vals8.view(np.uint8)
    mpad = np.zeros((OBJ_N, 2, B_PAD), dtype=f8)
    mpad[:, 0, :BANK_N] = masks[:, 0].astype(f8)
    mpad[:, 1, :BANK_N] = 1.0
    mr = mpad.reshape(OBJ_N, 2, NPAIR, 2, P)     # [o, row(0=mask,1=ones), pc, i, p]
    mo8 = np.zeros((OBJ_N, P, NPAIR, 32), dtype=f8)
    # [pc, i, j] blocks at stride 16: j=0 mask, j=1 ones
    mo8.reshape(OBJ_N, P, NPAIR, 2, 16)[:, :, :, :, 0:2] = (
        mr.transpose(0, 4, 2, 3, 1))
    mo8 = np.ascontiguousarray(mo8).reshape(OBJ_N, P, NPAIR * 32)
    mo8 = mo8.view(np.uint8)
    zeros8 = np.zeros((P, NWP), dtype=np.uint8)

    consts = np.zeros((2, P), dtype=bf)
    consts[1, :] = 1.0
    q_in_bf = q_in.astype(bf)
    q_out_bf = q_out.astype(bf)

    in_maps = []
    for core in range(N_CORES):
        o, half = divmod(core, 2)
        nsl = slice(half * N_HALF, (half + 1) * N_HALF)
        in_maps.append({
            "consts": consts,
            "keys": keys_pad[o],
            "vals": vals8[o],
            "mo": mo8[o],
            "zeros": zeros8,
            "qin": np.ascontiguousarray(q_in_bf[0, :, nsl]),
            "qout": np.ascontiguousarray(q_out_bf[0, :, nsl]),
        })

    run = _get_runner()
    results = run(in_maps)

    out = np.empty((1, OBJ_N, 2 * D_VAL, N_Q), dtype=np.float32)
    for core in range(N_CORES):
        o, half = divmod(core, 2)
        nsl = slice(half * N_HALF, (half + 1) * N_HALF)
        out[0, o, :, nsl] = results[core]["out"].astype(np.float32)
    return out
